# revision 38
# baseline (speedup 1.0000x reference)
"""Trainium2 Bass kernel for BetterPixelBilateralFilter2.

Problem: 5x5 dilated (dilation=3) bilateral filter over [B=2, C=32, 720, 1280]
with per-pixel range coefficients pc = -exp(coeffs)*softplus(scale) and
per-pixel spatial coefficients psy/psx.  Output = first 3 filtered channels.

Sharding: 8 cores = batch(2) x W-quarter(4).  Each core handles a full-height
[720, 320] slab of one batch image.

Device layout (per core), 6 chunks of 120 rows (= 4 subchunks x 30):
  - channel stage: partitions = (subchunk g, channel c) = 4x32; free = (y, x).
    All tap shifts are free-dim view offsets.  Per tap-pair: diff (DVE),
    square (ACT), then BOTH signs' mul-by-pc fused in one DVE TT (sign dim =
    positive-stride view pair; s=0 is the "-" tap).
  - channel reduce: per y-row, a matmul with a shifted view of a constant
    selection matrix (lhsT[:, p] = 1 iff p == pixel_partition(g, y))
    accumulates both signs into a 2-bank PSUM tile in pixel layout:
    partition p <-> row y = 4*(p//16) + p%4, subchunk g = (p%16)//4.
  - pixel stage (software-pipelined one pair behind the channel stage so
    DVE never waits on the reduce): exp from PSUM (ACT), spatial-map mul +
    t3 muls (DVE, sign-fused), then den and num accumulate in PSUM via
    identity matmuls on the PE (num as 2 flat 480-wide MMs per sign across
    2 banks).  Epilogue: den+1 (ACT bias), reciprocal, 4 piecewise flat
    muls, DMA out.  8 hole partitions are dropped on the host.
Perf notes (HW-measured): DVE is the wall (~1.27ms busy of ~1.38ms wall);
PE is co-saturated at its sustained ~1.2GHz rate (~330ns per 320-col MM,
thermal throttle active ~40% of the time).  GPSIMD shares the DVE SBUF
port (tested: useless).  Negative-stride AP dims and ACT reads spanning
two PSUM banks both hard-crash the exec unit (NRT status 101) though
CoreSim accepts them.

Border handling: host pads f with 1e4; (f - 1e4)^2 * pc <= -3e4 so exp
underflows to exactly 0 -- out-of-image taps contribute nothing.
"""

import numpy as np
import ml_dtypes

BF16 = ml_dtypes.bfloat16
PADV = 1.0e4

B, C, H, W = 2, 32, 720, 1280
NCORE = 8
WQ = 320           # x-quarter width per core
CH = 120           # rows per chunk
NG = 4             # y-subchunks per chunk
NY = 30            # rows per subchunk
NCH = H // CH      # 6 chunks
FH, FW = NY + 12, WQ + 24      # f-tile window 42 x 344 (rows 6..48 of pad-12)
D2H, D2W = 36, 326             # max diff-window (30+6, 320+6)
PXW = WQ + 12                  # f3 x-window 332
S0 = 113                       # selection-matrix center column
MW = S0 + 128                  # master selection matrix width

# positive tap offsets (dy,dx); each also covers its negation
POS = [(0, 1), (0, 2),
       (1, -2), (1, -1), (1, 0), (1, 1), (1, 2),
       (2, -2), (2, -1), (2, 0), (2, 1), (2, 2)]
SPKEYS = [(0, 1), (0, 4), (1, 0), (1, 1), (1, 4), (4, 0), (4, 1), (4, 4)]
SPIDX = {k: i for i, k in enumerate(SPKEYS)}


def _pixel_perm():
    """pperm[p] = chunk-local row (30*g + y_sub) for real partitions, -1 holes."""
    pperm = np.full(128, -1, np.int64)
    for y in range(NY):
        h, r = divmod(y, 4)
        for g in range(NG):
            pperm[16 * h + 4 * g + r] = NY * g + y
    return pperm


PPERM = _pixel_perm()          # [128], -1 at 8 hole slots
REAL = PPERM >= 0


def build_nc(n_chunks=NCH):
    import concourse.bacc as bacc
    import concourse.bass as bass
    import concourse.tile as tile
    from concourse import mybir

    def bcast_mid(a, n):
        """[P, X] view -> [P, n, X] with a stride-0 middle dim."""
        return bass.AP(tensor=a.tensor, offset=a.offset,
                       ap=[a.ap[0], [0, n], a.ap[1]])

    def with_dim(a, pos, pair):
        """Insert a [stride, n] dim at free position pos (0 = outermost
        free dim) of AP a. Strides are in elements."""
        i = pos + 1   # skip the partition dim
        return bass.AP(tensor=a.tensor, offset=a.offset,
                       ap=a.ap[:i] + [list(pair)] + a.ap[i:])

    bf = mybir.dt.bfloat16
    f32 = mybir.dt.float32
    AF = mybir.ActivationFunctionType
    OP = mybir.AluOpType

    nc = bacc.Bacc("TRN2", num_devices=NCORE, debug=False)
    fin = nc.dram_tensor("fin", [n_chunks, 128, FH, FW], bf,
                         kind="ExternalInput").ap()
    pcin = nc.dram_tensor("pcin", [n_chunks, 128, NY, WQ], bf,
                          kind="ExternalInput").ap()
    f3in = nc.dram_tensor("f3in", [n_chunks, 128, 5, 3, PXW], bf,
                          kind="ExternalInput").ap()
    lgspin = nc.dram_tensor("lgspin", [n_chunks, 128, 8, WQ], bf,
                            kind="ExternalInput").ap()
    selin = nc.dram_tensor("selin", [128, MW], bf, kind="ExternalInput").ap()
    idin = nc.dram_tensor("idin", [128, 128], bf, kind="ExternalInput").ap()
    out = nc.dram_tensor("out", [n_chunks, 128, 3, WQ], f32,
                         kind="ExternalOutput").ap()

    with tile.TileContext(nc) as tc:
        with (
            tc.tile_pool(name="consts", bufs=1) as consts,
            tc.tile_pool(name="fload", bufs=1) as fload,
            tc.tile_pool(name="pxload", bufs=1) as pxload,
            tc.tile_pool(name="dpool", bufs=2) as dpool,
            tc.tile_pool(name="prpool", bufs=2) as prpool,
            tc.tile_pool(name="wpool", bufs=2) as wpool,
            tc.tile_pool(name="opool", bufs=1) as opool,
            tc.tile_pool(name="lwpool", bufs=2, space="PSUM") as lwpool,
            tc.tile_pool(name="acpool", bufs=1, space="PSUM") as acpool,
        ):
            selt = consts.tile([128, MW], bf)
            nc.sync.dma_start(out=selt, in_=selin)
            idt = consts.tile([128, 128], bf)
            nc.sync.dma_start(out=idt, in_=idin)

            for j in range(n_chunks):
                ft = fload.tile([128, FH, FW], bf, tag="ft")
                pct = fload.tile([128, NY, WQ], bf, tag="pct")
                f3t = pxload.tile([128, 5, 3, PXW], bf, tag="f3t")
                lgt = pxload.tile([128, 8, WQ], bf, tag="lgt")
                nc.sync.dma_start(out=ft, in_=fin[j])
                nc.sync.dma_start(out=pct, in_=pcin[j])
                nc.sync.dma_start(out=f3t, in_=f3in[j])
                nc.sync.dma_start(out=lgt, in_=lgspin[j])

                # per-chunk PSUM accumulators: den [WQ] + num as a flat
                # (k, x) layout split across 2 banks of 480 useful f32 each
                # (center tap's den contribution of 1.0 is added as an ACT
                # bias in the epilogue; num's center init is these 4 MMs)
                dent = acpool.tile([128, WQ], f32, tag="den")
                numt = acpool.tile([128, 2, 512], f32, tag="num")

                def flush_tail(pend):
                    """exp/t3/den/num for a finished pair."""
                    ip, dy, dx, lwp = pend
                    last = ip == len(POS) - 1
                    # exp -> weights (spatial already in the PSUM init); one
                    # ACT inst per sign (one PSUM bank per instruction)
                    wfs = wpool.tile([128, 2, WQ], bf, tag="wfs")
                    for s in range(2):
                        nc.scalar.activation(out=wfs[:, s], in_=lwp[:, s, :WQ],
                                             func=AF.Exp)
                    # num contributions: t3[s,k,x] = wfs[s,x]*f3(nb_s).
                    # s=0 is the "-" tap (smaller f3t offset, matching prodt
                    # sign order), so the sign stride is positive here too.
                    t3s = wpool.tile([128, 2, 3, WQ], bf, tag="t3s")
                    f3m = f3t[:, 2 - dy, :, 6 - 3 * dx:6 - 3 * dx + WQ]
                    f3step = (2 * dy) * (3 * PXW) + (6 * dx)
                    assert f3step > 0
                    f3pair = with_dim(f3m, 0, (f3step, 2))
                    wpair = with_dim(wfs[:], 1, (0, 3))
                    nc.vector.tensor_mul(out=t3s, in0=wpair, in1=f3pair)
                    # PSUM-accumulate den and num on the PE; num as 2 flat
                    # 480-element MMs per sign (t3s free layout (k,x) is
                    # contiguous per sign)
                    t3a = t3s[:]
                    for s in range(2):
                        nc.tensor.matmul(out=dent, lhsT=idt, rhs=wfs[:, s],
                                         start=(ip == 0 and s == 0),
                                         stop=last and s == 1)
                        for h in range(2):
                            rhs = bass.AP(tensor=t3a.tensor,
                                          offset=t3a.offset + s * 960 + h * 480,
                                          ap=[t3a.ap[0], [1, 480]])
                            nc.tensor.matmul(out=numt[:, h, 0:480], lhsT=idt,
                                             rhs=rhs,
                                             start=(ip == 0 and s == 0),
                                             stop=last and s == 1)
                    if ip == 0:
                        # center tap (w=1) accumulated after the first
                        # start=True MMs reset the banks
                        nc.tensor.matmul(out=numt[:, 0, 0:WQ], lhsT=idt,
                                         rhs=f3t[:, 2, 0, 6:6 + WQ],
                                         start=False, stop=False)
                        nc.tensor.matmul(out=numt[:, 0, WQ:480], lhsT=idt,
                                         rhs=f3t[:, 2, 1, 6:6 + 160],
                                         start=False, stop=False)
                        nc.tensor.matmul(out=numt[:, 1, 0:160], lhsT=idt,
                                         rhs=f3t[:, 2, 1, 166:166 + 160],
                                         start=False, stop=False)
                        nc.tensor.matmul(out=numt[:, 1, 160:480], lhsT=idt,
                                         rhs=f3t[:, 2, 2, 6:6 + WQ],
                                         start=False, stop=False)

                for ip, (dy, dx) in enumerate(POS):
                    y0 = -3 * dy                  # <= 0
                    x0w = min(0, -3 * dx)
                    wy = NY + 3 * dy
                    wx = WQ + 3 * abs(dx)
                    dft = dpool.tile([128, D2H, D2W], bf, tag="dft")
                    dv = dft[:, :wy, :wx]
                    i0y, i0x = 6 + y0, 12 + x0w
                    i1y, i1x = 6 + y0 + 3 * dy, 12 + x0w + 3 * dx
                    in0 = ft[:, i0y:i0y + wy, i0x:i0x + wx]
                    in1 = ft[:, i1y:i1y + wy, i1x:i1x + wx]
                    # GPSIMD is useless here: its SBUF port is an exclusive
                    # lock shared with DVE, so GPSIMD tensor ops stall DVE.
                    # (Odd element offsets still get DVE 2x on this silicon.)
                    nc.vector.tensor_sub(out=dv, in0=in0, in1=in1)
                    nc.scalar.activation(out=dv, in_=dv, func=AF.Square)

                    # both signs' pc*d2 in ONE fused TT instruction.
                    # s=0 is the "-" tap (smaller dft offset), s=1 the "+"
                    # tap, so the sign-dim stride is always positive (HW AP
                    # steps may not support negative outer strides).
                    ry_p, rx_p = 3 * dy, max(0, 3 * dx)
                    ry_m, rx_m = 0, max(0, -3 * dx)
                    prodt = prpool.tile([128, 2, NY, WQ], bf, tag="prod")
                    d2m = dft[:, ry_m:ry_m + NY, rx_m:rx_m + WQ]
                    sgn_step = ((ry_p - ry_m) * D2W + (rx_p - rx_m))
                    assert sgn_step > 0
                    d2pair = with_dim(d2m, 0, (sgn_step, 2))
                    pcpair = with_dim(pct[:], 0, (0, 2))
                    nc.vector.tensor_mul(out=prodt, in0=pcpair, in1=d2pair)

                    m = SPIDX[(dy * dy, dx * dx)]
                    # [2, 512] so each sign's slice is bank-aligned (2KB)
                    lwp = lwpool.tile([128, 2, 512], f32, tag="lw")
                    # init both signs' PSUM with the log-spatial map
                    for s in range(2):
                        nc.tensor.matmul(out=lwp[:, s, :WQ], lhsT=idt,
                                         rhs=lgt[:, m], start=True,
                                         stop=False)
                    # interleave the two taps' matmuls y-major so adjacent
                    # matmuls share the same stationary selection view
                    for y in range(NY):
                        sy = S0 - (16 * (y // 4) + (y % 4))
                        for s in range(2):
                            nc.tensor.matmul(
                                out=lwp[:, s, :WQ],
                                lhsT=selt[:, sy:sy + 128],
                                rhs=prodt[:, s, y, :],
                                start=False, stop=(y == NY - 1),
                            )
                    flush_tail((ip, dy, dx, lwp))

                # den += 1.0 (center tap), then reciprocal
                den1 = opool.tile([128, WQ], f32, tag="den1")
                nc.scalar.activation(out=den1, in_=dent, func=AF.Identity,
                                     bias=1.0)
                rden = opool.tile([128, WQ], f32, tag="rden")
                nc.vector.reciprocal(out=rden, in_=den1)
                # num lives flat (k,x) across 2 banks of 480; multiply by
                # rden piecewise at channel boundaries
                outt = opool.tile([128, 3, WQ], f32, tag="outt")
                rd, oa, na = rden[:], outt[:], numt[:]

                def flat(a, off, n):
                    return bass.AP(tensor=a.tensor, offset=a.offset + off,
                                   ap=[a.ap[0], [1, n]])

                # flat num f = k*320+x lives at bank f//480, off f%480
                for (boff, ooff, roff, n) in ((0, 0, 0, 320),
                                              (320, 320, 0, 160),
                                              (512, 480, 160, 160),
                                              (512 + 160, 640, 0, 320)):
                    nc.vector.tensor_mul(out=flat(oa, ooff, n),
                                         in0=flat(na, boff, n),
                                         in1=flat(rd, roff, n))
                nc.sync.dma_start(out=out[j], in_=outt)

    nc.compile()
    return nc


def prep_inputs(input, coeffs, n_chunks=NCH):
    """Build per-core in_maps (list of 8 dicts of numpy arrays)."""
    inp = np.asarray(input, np.float32)
    f = inp[:, :C]                      # [2,32,720,1280]
    scale = inp[:, C:]                  # [2,34,720,1280]
    k = np.exp(np.asarray(coeffs, np.float32).reshape(-1))   # [34]
    sp = np.logaddexp(0.0, scale)
    params = -(k[None, :, None, None] * sp)
    pc = params[:, :C]
    psy = params[:, C]                  # [2,720,1280]
    psx = params[:, C + 1]

    # padded f: rows/cols shifted by +12
    fp = np.full((B, C, H + 24, W + 24), PADV, np.float32)
    fp[:, :, 12:12 + H, 12:12 + W] = f
    # padded first-3-channel f for the pixel stage: shifted by +6
    f3p = np.full((B, 3, H + 12, W + 12), PADV, np.float32)
    f3p[:, :, 6:6 + H, 6:6 + W] = f[:, :3]

    # log-spatial maps psy*dy2 + psx*dx2 (added into PSUM pre-exp)
    spmaps = np.empty((B, 8, H, W), np.float32)
    for i, (a2, b2) in enumerate(SPKEYS):
        spmaps[:, i] = psy * a2 + psx * b2

    # selection master matrix: sel[(g,c), v] = 1 iff v == S0 + 4g
    sel = np.zeros((128, MW), np.float32)
    for g in range(NG):
        sel[32 * g:32 * (g + 1), S0 + 4 * g] = 1.0
    ident = np.eye(128, dtype=np.float32)

    # row-gather index with holes -> clamp to row 0 and zero later
    prow = np.where(REAL, PPERM, 0)

    in_maps = []
    for b in range(B):
        for q in range(4):
            x0 = WQ * q
            # window rows 6..48 of the pad-12 frame (rows 0-5/48-53 unused)
            fpb = fp[b, :, 6:, x0:x0 + FW]         # [32, 738, 344]
            s = fpb.strides
            fin = np.lib.stride_tricks.as_strided(
                fpb, shape=(n_chunks, NG, C, FH, FW),
                strides=(CH * s[1], NY * s[1], s[0], s[1], s[2]),
            ).reshape(n_chunks, 128, FH, FW)

            pcb = pc[b, :, :, x0:x0 + WQ]          # [32, 720, 320]
            s = pcb.strides
            pcin = np.lib.stride_tricks.as_strided(
                pcb, shape=(n_chunks, NG, C, NY, WQ),
                strides=(CH * s[1], NY * s[1], s[0], s[1], s[2]),
            ).reshape(n_chunks, 128, NY, WQ)

            # f3in[j, d, p, c, xx] = f3p[b, c, 120j + prow[p] + 3(d-2) + 6, x0+xx]
            j_idx = np.arange(n_chunks)[:, None, None]
            d_idx = np.arange(5)[None, :, None]
            p_idx = prow[None, None, :]
            rows = CH * j_idx + p_idx + 3 * (d_idx - 2) + 6   # [j, d, p]
            f3in = f3p[b][:, rows, x0:x0 + PXW]               # [3, j, d, p, PXW]
            # -> [j, p, d, c, x] to match SBUF tile [128, 5, 3, PXW]
            f3in = np.ascontiguousarray(f3in.transpose(1, 3, 2, 0, 4))
            f3in[:, ~REAL] = 0.0

            # lgspin[j, p, m, xx] = spmaps[b, m, 120j + prow[p], x0+xx]
            rows2 = CH * np.arange(n_chunks)[:, None] + prow[None, :]  # [j, p]
            spin = spmaps[b][:, rows2, x0:x0 + WQ]            # [8, j, p, WQ]
            spin = np.ascontiguousarray(spin.transpose(1, 2, 0, 3))
            spin[:, ~REAL] = 0.0

            in_maps.append({
                "fin": fin.astype(BF16),
                "pcin": pcin.astype(BF16),
                "f3in": f3in.astype(BF16),
                "lgspin": spin.astype(BF16),
                "selin": sel.astype(BF16),
                "idin": ident.astype(BF16),
            })
    return in_maps


def assemble_output(results, n_chunks=NCH):
    outf = np.empty((B, 3, H, W), np.float32)
    i = 0
    for b in range(B):
        for q in range(4):
            x0 = WQ * q
            o = np.asarray(results[i]["out"], np.float32)  # [j, 128, 3, WQ]
            for j in range(n_chunks):
                # fancy-index on axis 2 with slice on axis 1 -> result axes
                # are (row, c, x), matching o[j, REAL] directly
                outf[b, :, CH * j + PPERM[REAL], x0:x0 + WQ] = o[j, REAL]
            i += 1
    return outf


_NC_CACHE = {}


def kernel(input, coeffs, kernel_size=5, dilation=3, dynamic_size=3):
    assert int(kernel_size) == 5 and int(dilation) == 3
    assert int(dynamic_size) == 3
    from concourse import bass_utils

    if "nc" not in _NC_CACHE:
        _NC_CACHE["nc"] = build_nc(NCH)
    nc = _NC_CACHE["nc"]
    in_maps = prep_inputs(input, coeffs, NCH)
    res = bass_utils.run_bass_kernel_spmd(nc, in_maps,
                                          core_ids=list(range(NCORE)))
    return assemble_output(res.results, NCH)



# revision 45
# speedup vs baseline: 1.1856x; 1.1856x over previous
"""Trainium2 Bass kernel for BetterPixelBilateralFilter2.

Problem: 5x5 dilated (dilation=3) bilateral filter over [B=2, C=32, 720, 1280]
with per-pixel range coefficients pc = -exp(coeffs)*softplus(scale) and
per-pixel spatial coefficients psy/psx.  Output = first 3 filtered channels.

Sharding: 8 cores = batch(2) x W-quarter(4).  Each core handles a full-height
[720, 320] slab of one batch image.

Device layout (per core), 6 chunks of 120 rows (= 4 subchunks x 30):
  - channel stage: partitions = (subchunk g, channel c) = 4x32; free = (y, x).
    All tap shifts are free-dim view offsets.  Per tap-pair: diff (DVE),
    square (ACT), then BOTH signs' mul-by-pc fused in one DVE TT (sign dim =
    positive-stride view pair; s=0 is the "-" tap).
  - channel reduce: per y-row, a matmul with a shifted view of a constant
    selection matrix (lhsT[:, p] = 1 iff p == pixel_partition(g, y))
    accumulates both signs into a 2-bank PSUM tile in pixel layout:
    partition p <-> row y = 4*(p//16) + p%4, subchunk g = (p%16)//4.
  - pixel stage (software-pipelined one pair behind the channel stage so
    DVE never waits on the reduce): exp from PSUM (ACT), spatial-map mul +
    t3 muls (DVE, sign-fused), then den and num accumulate in PSUM via
    identity matmuls on the PE (num as 2 flat 480-wide MMs per sign across
    2 banks).  Epilogue: den+1 (ACT bias), reciprocal, 4 piecewise flat
    muls, DMA out.  8 hole partitions are dropped on the host.
Perf notes (HW-measured): DVE is the wall (~1.27ms busy of ~1.38ms wall);
PE is co-saturated at its sustained ~1.2GHz rate (~330ns per 320-col MM,
thermal throttle active ~40% of the time).  GPSIMD shares the DVE SBUF
port (tested: useless).  Negative-stride AP dims and ACT reads spanning
two PSUM banks both hard-crash the exec unit (NRT status 101) though
CoreSim accepts them.

Border handling: host pads f with 1e4; (f - 1e4)^2 * pc <= -3e4 so exp
underflows to exactly 0 -- out-of-image taps contribute nothing.
"""

import numpy as np
import ml_dtypes

BF16 = ml_dtypes.bfloat16
PADV = 1.0e4

B, C, H, W = 2, 32, 720, 1280
NCORE = 8
WQ = 320           # x-quarter width per core
CH = 120           # rows per chunk
NG = 4             # y-subchunks per chunk
NY = 30            # rows per subchunk
NCH = H // CH      # 6 chunks
FH, FW = NY + 12, WQ + 24      # f-tile window 42 x 344 (rows 6..48 of pad-12)
D2H, D2W = 36, 326             # max diff-window (30+6, 320+6)
PXW = WQ + 12                  # f3 x-window 332
S0 = 113                       # selection-matrix center column
MW = S0 + 128                  # master selection matrix width

# positive tap offsets (dy,dx); each also covers its negation
POS = [(0, 1), (0, 2),
       (1, -2), (1, -1), (1, 0), (1, 1), (1, 2),
       (2, -2), (2, -1), (2, 0), (2, 1), (2, 2)]
SPKEYS = [(0, 1), (0, 4), (1, 0), (1, 1), (1, 4), (4, 0), (4, 1), (4, 4)]
SPIDX = {k: i for i, k in enumerate(SPKEYS)}


def _pixel_perm():
    """pperm[p] = chunk-local row (30*g + y_sub) for real partitions, -1 holes."""
    pperm = np.full(128, -1, np.int64)
    for y in range(NY):
        h, r = divmod(y, 4)
        for g in range(NG):
            pperm[16 * h + 4 * g + r] = NY * g + y
    return pperm


PPERM = _pixel_perm()          # [128], -1 at 8 hole slots
REAL = PPERM >= 0


def build_nc(n_chunks=NCH):
    import concourse.bacc as bacc
    import concourse.bass as bass
    import concourse.tile as tile
    from concourse import mybir

    def bcast_mid(a, n):
        """[P, X] view -> [P, n, X] with a stride-0 middle dim."""
        return bass.AP(tensor=a.tensor, offset=a.offset,
                       ap=[a.ap[0], [0, n], a.ap[1]])

    def with_dim(a, pos, pair):
        """Insert a [stride, n] dim at free position pos (0 = outermost
        free dim) of AP a. Strides are in elements."""
        i = pos + 1   # skip the partition dim
        return bass.AP(tensor=a.tensor, offset=a.offset,
                       ap=a.ap[:i] + [list(pair)] + a.ap[i:])

    bf = mybir.dt.bfloat16
    f32 = mybir.dt.float32
    AF = mybir.ActivationFunctionType
    OP = mybir.AluOpType

    nc = bacc.Bacc("TRN2", num_devices=NCORE, debug=False)
    fin = nc.dram_tensor("fin", [n_chunks, 128, FH, FW], bf,
                         kind="ExternalInput").ap()
    pcin = nc.dram_tensor("pcin", [n_chunks, 128, NY, WQ], bf,
                          kind="ExternalInput").ap()
    f3in = nc.dram_tensor("f3in", [n_chunks, 128, 5, 3, PXW], bf,
                          kind="ExternalInput").ap()
    spin = nc.dram_tensor("spin", [n_chunks, 128, 8, WQ], bf,
                          kind="ExternalInput").ap()
    selin = nc.dram_tensor("selin", [128, MW], bf, kind="ExternalInput").ap()
    idin = nc.dram_tensor("idin", [128, 128], bf, kind="ExternalInput").ap()
    out = nc.dram_tensor("out", [n_chunks, 128, 3, WQ], f32,
                         kind="ExternalOutput").ap()

    with tile.TileContext(nc) as tc:
        with (
            tc.tile_pool(name="consts", bufs=1) as consts,
            tc.tile_pool(name="fload", bufs=1) as fload,
            tc.tile_pool(name="pxload", bufs=1) as pxload,
            tc.tile_pool(name="dpool", bufs=2) as dpool,
            tc.tile_pool(name="prpool", bufs=2) as prpool,
            tc.tile_pool(name="wpool", bufs=2) as wpool,
            tc.tile_pool(name="opool", bufs=1) as opool,
            tc.tile_pool(name="lwpool", bufs=2, space="PSUM") as lwpool,
            tc.tile_pool(name="acpool", bufs=1, space="PSUM") as acpool,
        ):
            selt = consts.tile([128, MW], bf)
            nc.sync.dma_start(out=selt, in_=selin)
            idt = consts.tile([128, 128], bf)
            nc.sync.dma_start(out=idt, in_=idin)

            for j in range(n_chunks):
                ft = fload.tile([128, FH, FW], bf, tag="ft")
                pct = fload.tile([128, NY, WQ], bf, tag="pct")
                f3t = pxload.tile([128, 5, 3, PXW], bf, tag="f3t")
                spt = pxload.tile([128, 8, WQ], bf, tag="spt")
                nc.sync.dma_start(out=ft, in_=fin[j])
                nc.sync.dma_start(out=pct, in_=pcin[j])
                nc.sync.dma_start(out=f3t, in_=f3in[j])
                nc.sync.dma_start(out=spt, in_=spin[j])

                # per-chunk PSUM accumulators: den [WQ] + num as a flat
                # (k, x) layout split across 2 banks of 480 useful f32 each
                # (center tap's den contribution of 1.0 is added as an ACT
                # bias in the epilogue; num's center init is these 4 MMs)
                dent = acpool.tile([128, WQ], f32, tag="den")
                numt = acpool.tile([128, 2, 512], f32, tag="num")

                def flush_tail(pend):
                    """exp/spatial/t3/den/num for a finished pair (deferred
                    one iteration so DVE never waits on the PE's reduce)."""
                    ip, dy, dx, lwp = pend
                    last = ip == len(POS) - 1
                    m = SPIDX[(dy * dy, dx * dx)]
                    # exp -> range weights; one ACT inst per sign (one PSUM
                    # bank per instruction)
                    wts = wpool.tile([128, 2, WQ], bf, tag="wts")
                    for s in range(2):
                        nc.scalar.activation(out=wts[:, s], in_=lwp[:, s, :WQ],
                                             func=AF.Exp)
                    # * spatial weight (same map for both signs)
                    wfs = wpool.tile([128, 2, WQ], bf, tag="wfs")
                    nc.vector.tensor_mul(out=wfs, in0=wts,
                                         in1=with_dim(spt[:, m], 0, (0, 2)))
                    # num contributions: t3[s,k,x] = wfs[s,x]*f3(nb_s).
                    # s=0 is the "-" tap (smaller f3t offset, matching prodt
                    # sign order), so the sign stride is positive here too.
                    t3s = wpool.tile([128, 2, 3, WQ], bf, tag="t3s")
                    f3m = f3t[:, 2 - dy, :, 6 - 3 * dx:6 - 3 * dx + WQ]
                    f3step = (2 * dy) * (3 * PXW) + (6 * dx)
                    assert f3step > 0
                    f3pair = with_dim(f3m, 0, (f3step, 2))
                    wpair = with_dim(wfs[:], 1, (0, 3))
                    nc.vector.tensor_mul(out=t3s, in0=wpair, in1=f3pair)
                    # PSUM-accumulate den and num on the PE; num as 2 flat
                    # 480-element MMs per sign (t3s free layout (k,x) is
                    # contiguous per sign)
                    t3a = t3s[:]
                    for s in range(2):
                        nc.tensor.matmul(out=dent, lhsT=idt, rhs=wfs[:, s],
                                         start=(ip == 0 and s == 0),
                                         stop=last and s == 1)
                        for h in range(2):
                            rhs = bass.AP(tensor=t3a.tensor,
                                          offset=t3a.offset + s * 960 + h * 480,
                                          ap=[t3a.ap[0], [1, 480]])
                            nc.tensor.matmul(out=numt[:, h, 0:480], lhsT=idt,
                                             rhs=rhs,
                                             start=(ip == 0 and s == 0),
                                             stop=last and s == 1)
                    if ip == 0:
                        # center tap (w=1) accumulated after the first
                        # start=True MMs reset the banks
                        nc.tensor.matmul(out=numt[:, 0, 0:WQ], lhsT=idt,
                                         rhs=f3t[:, 2, 0, 6:6 + WQ],
                                         start=False, stop=False)
                        nc.tensor.matmul(out=numt[:, 0, WQ:480], lhsT=idt,
                                         rhs=f3t[:, 2, 1, 6:6 + 160],
                                         start=False, stop=False)
                        nc.tensor.matmul(out=numt[:, 1, 0:160], lhsT=idt,
                                         rhs=f3t[:, 2, 1, 166:166 + 160],
                                         start=False, stop=False)
                        nc.tensor.matmul(out=numt[:, 1, 160:480], lhsT=idt,
                                         rhs=f3t[:, 2, 2, 6:6 + WQ],
                                         start=False, stop=False)

                pending = None
                for ip, (dy, dx) in enumerate(POS):
                    y0 = -3 * dy                  # <= 0
                    x0w = min(0, -3 * dx)
                    wy = NY + 3 * dy
                    wx = WQ + 3 * abs(dx)
                    dft = dpool.tile([128, D2H, D2W], bf, tag="dft")
                    dv = dft[:, :wy, :wx]
                    i0y, i0x = 6 + y0, 12 + x0w
                    i1y, i1x = 6 + y0 + 3 * dy, 12 + x0w + 3 * dx
                    in0 = ft[:, i0y:i0y + wy, i0x:i0x + wx]
                    in1 = ft[:, i1y:i1y + wy, i1x:i1x + wx]
                    # GPSIMD is useless here: its SBUF port is an exclusive
                    # lock shared with DVE, so GPSIMD tensor ops stall DVE.
                    # (Odd element offsets still get DVE 2x on this silicon.)
                    nc.vector.tensor_sub(out=dv, in0=in0, in1=in1)
                    nc.scalar.activation(out=dv, in_=dv, func=AF.Square)

                    # both signs' pc*d2 in ONE fused TT instruction.
                    # s=0 is the "-" tap (smaller dft offset), s=1 the "+"
                    # tap, so the sign-dim stride is always positive (HW AP
                    # steps may not support negative outer strides).
                    ry_p, rx_p = 3 * dy, max(0, 3 * dx)
                    ry_m, rx_m = 0, max(0, -3 * dx)
                    prodt = prpool.tile([128, 2, NY, WQ], bf, tag="prod")
                    d2m = dft[:, ry_m:ry_m + NY, rx_m:rx_m + WQ]
                    sgn_step = ((ry_p - ry_m) * D2W + (rx_p - rx_m))
                    assert sgn_step > 0
                    d2pair = with_dim(d2m, 0, (sgn_step, 2))
                    pcpair = with_dim(pct[:], 0, (0, 2))
                    nc.vector.tensor_mul(out=prodt, in0=pcpair, in1=d2pair)

                    # [2, 512] so each sign's slice is bank-aligned (2KB)
                    lwp = lwpool.tile([128, 2, 512], f32, tag="lw")
                    # interleave the two taps' matmuls y-major so adjacent
                    # matmuls share the same stationary selection view
                    for y in range(NY):
                        sy = S0 - (16 * (y // 4) + (y % 4))
                        for s in range(2):
                            nc.tensor.matmul(
                                out=lwp[:, s, :WQ],
                                lhsT=selt[:, sy:sy + 128],
                                rhs=prodt[:, s, y, :],
                                start=(y == 0), stop=(y == NY - 1),
                            )
                    if pending is not None:
                        flush_tail(pending)
                    pending = (ip, dy, dx, lwp)
                flush_tail(pending)

                # den += 1.0 (center tap), then reciprocal
                den1 = opool.tile([128, WQ], f32, tag="den1")
                nc.scalar.activation(out=den1, in_=dent, func=AF.Identity,
                                     bias=1.0)
                rden = opool.tile([128, WQ], f32, tag="rden")
                nc.vector.reciprocal(out=rden, in_=den1)
                # num lives flat (k,x) across 2 banks of 480; multiply by
                # rden piecewise at channel boundaries
                outt = opool.tile([128, 3, WQ], f32, tag="outt")
                rd, oa, na = rden[:], outt[:], numt[:]

                def flat(a, off, n):
                    return bass.AP(tensor=a.tensor, offset=a.offset + off,
                                   ap=[a.ap[0], [1, n]])

                # flat num f = k*320+x lives at bank f//480, off f%480
                for (boff, ooff, roff, n) in ((0, 0, 0, 320),
                                              (320, 320, 0, 160),
                                              (512, 480, 160, 160),
                                              (512 + 160, 640, 0, 320)):
                    nc.vector.tensor_mul(out=flat(oa, ooff, n),
                                         in0=flat(na, boff, n),
                                         in1=flat(rd, roff, n))
                nc.sync.dma_start(out=out[j], in_=outt)

    nc.compile()
    return nc


def prep_inputs(input, coeffs, n_chunks=NCH):
    """Build per-core in_maps (list of 8 dicts of numpy arrays)."""
    inp = np.asarray(input, np.float32)
    f = inp[:, :C]                      # [2,32,720,1280]
    scale = inp[:, C:]                  # [2,34,720,1280]
    k = np.exp(np.asarray(coeffs, np.float32).reshape(-1))   # [34]
    sp = np.logaddexp(0.0, scale)
    params = -(k[None, :, None, None] * sp)
    pc = params[:, :C]
    psy = params[:, C]                  # [2,720,1280]
    psx = params[:, C + 1]

    # padded f: rows/cols shifted by +12
    fp = np.full((B, C, H + 24, W + 24), PADV, np.float32)
    fp[:, :, 12:12 + H, 12:12 + W] = f
    # padded first-3-channel f for the pixel stage: shifted by +6
    f3p = np.full((B, 3, H + 12, W + 12), PADV, np.float32)
    f3p[:, :, 6:6 + H, 6:6 + W] = f[:, :3]

    # spatial maps exp(psy*dy2 + psx*dx2)
    spmaps = np.empty((B, 8, H, W), np.float32)
    for i, (a2, b2) in enumerate(SPKEYS):
        spmaps[:, i] = np.exp(psy * a2 + psx * b2)

    # selection master matrix: sel[(g,c), v] = 1 iff v == S0 + 4g
    sel = np.zeros((128, MW), np.float32)
    for g in range(NG):
        sel[32 * g:32 * (g + 1), S0 + 4 * g] = 1.0
    ident = np.eye(128, dtype=np.float32)

    # row-gather index with holes -> clamp to row 0 and zero later
    prow = np.where(REAL, PPERM, 0)

    in_maps = []
    for b in range(B):
        for q in range(4):
            x0 = WQ * q
            # window rows 6..48 of the pad-12 frame (rows 0-5/48-53 unused)
            fpb = fp[b, :, 6:, x0:x0 + FW]         # [32, 738, 344]
            s = fpb.strides
            fin = np.lib.stride_tricks.as_strided(
                fpb, shape=(n_chunks, NG, C, FH, FW),
                strides=(CH * s[1], NY * s[1], s[0], s[1], s[2]),
            ).reshape(n_chunks, 128, FH, FW)

            pcb = pc[b, :, :, x0:x0 + WQ]          # [32, 720, 320]
            s = pcb.strides
            pcin = np.lib.stride_tricks.as_strided(
                pcb, shape=(n_chunks, NG, C, NY, WQ),
                strides=(CH * s[1], NY * s[1], s[0], s[1], s[2]),
            ).reshape(n_chunks, 128, NY, WQ)

            # f3in[j, d, p, c, xx] = f3p[b, c, 120j + prow[p] + 3(d-2) + 6, x0+xx]
            j_idx = np.arange(n_chunks)[:, None, None]
            d_idx = np.arange(5)[None, :, None]
            p_idx = prow[None, None, :]
            rows = CH * j_idx + p_idx + 3 * (d_idx - 2) + 6   # [j, d, p]
            f3in = f3p[b][:, rows, x0:x0 + PXW]               # [3, j, d, p, PXW]
            # -> [j, p, d, c, x] to match SBUF tile [128, 5, 3, PXW]
            f3in = np.ascontiguousarray(f3in.transpose(1, 3, 2, 0, 4))
            f3in[:, ~REAL] = 0.0

            # lgspin[j, p, m, xx] = spmaps[b, m, 120j + prow[p], x0+xx]
            rows2 = CH * np.arange(n_chunks)[:, None] + prow[None, :]  # [j, p]
            spin = spmaps[b][:, rows2, x0:x0 + WQ]            # [8, j, p, WQ]
            spin = np.ascontiguousarray(spin.transpose(1, 2, 0, 3))
            spin[:, ~REAL] = 0.0

            in_maps.append({
                "fin": fin.astype(BF16),
                "pcin": pcin.astype(BF16),
                "f3in": f3in.astype(BF16),
                "spin": spin.astype(BF16),
                "selin": sel.astype(BF16),
                "idin": ident.astype(BF16),
            })
    return in_maps


def assemble_output(results, n_chunks=NCH):
    outf = np.empty((B, 3, H, W), np.float32)
    i = 0
    for b in range(B):
        for q in range(4):
            x0 = WQ * q
            o = np.asarray(results[i]["out"], np.float32)  # [j, 128, 3, WQ]
            for j in range(n_chunks):
                # fancy-index on axis 2 with slice on axis 1 -> result axes
                # are (row, c, x), matching o[j, REAL] directly
                outf[b, :, CH * j + PPERM[REAL], x0:x0 + WQ] = o[j, REAL]
            i += 1
    return outf


_NC_CACHE = {}


def kernel(input, coeffs, kernel_size=5, dilation=3, dynamic_size=3):
    assert int(kernel_size) == 5 and int(dilation) == 3
    assert int(dynamic_size) == 3
    from concourse import bass_utils

    if "nc" not in _NC_CACHE:
        _NC_CACHE["nc"] = build_nc(NCH)
    nc = _NC_CACHE["nc"]
    in_maps = prep_inputs(input, coeffs, NCH)
    res = bass_utils.run_bass_kernel_spmd(nc, in_maps,
                                          core_ids=list(range(NCORE)))
    return assemble_output(res.results, NCH)

